# revision 1
# baseline (speedup 1.0000x reference)
"""Trainium2 Bass kernel for ModelToVolumeAligner — v2 (2D-binned, collective-free).

Strategy:
  - Host: rotate positions, clip to an 88^3 box centered on the cloud,
    2D-bin atoms by (y,x) into 8x8 bins of 11 voxels, shard atoms per bin
    round-robin across 8 cores (uniform compile-time layout via per-bin
    padding), precompute per-rank quadratic exp-arg params.
  - Device k1 (per core): per 128-rank tile, ONE small matmul (contract 9)
    against a fixed basis computes the exp-arguments for the z-profile W
    (88 cols) and the y/x window profiles u,v (17 cols each); batched ACT
    Exp over 4-tile chunks -> bf16. kr = u (x) v via DVE/Pool broadcast TT
    (or PE-arg + ACT Exp for some tiles). Splat: matmul contracting ranks:
    slab[z, 17x17 window] += W^T @ kr, accumulated in PSUM across x-paired
    bins; slabs TT-added into a bf16 vol [88, 94x94]. Interior [88,88,88]
    DMA'd out as the partial volume (bf16).
  - Host: restack 8 partial volumes into per-core z-shards.
  - Device k2 (per core): pairwise-tree add of the 8 partial z-shards,
    then ssq/dot reductions vs the clipped voxel grid -> [2] scalars.
  - Host: combine 8x2 scalars + host gssq -> 1 - dot/sqrt(ssq*gssq).
    (The v/v.sum() normalization cancels in the correlation.)
  - No collectives: ReduceScatter in this environment costs ~1ms fixed.
"""

import math
import numpy as np

import concourse.bass as bass
import concourse.mybir as mybir
import concourse.tile as tile
from concourse import bacc
from concourse.bass import ts
from concourse.bass_utils import run_bass_kernel_spmd

F32 = mybir.dt.float32
F32R = mybir.dt.float32r
F16 = mybir.dt.float16
BF16 = mybir.dt.bfloat16
AF = mybir.ActivationFunctionType
OP = mybir.AluOpType

N_CORES = 8
P = 128
G = 5
BOX = 88            # clipped cube side (voxels)
NB = 8              # bins per axis
BINV = BOX // NB    # 11 voxels per bin
MARGIN = 3
WIN = BINV + 2 * MARGIN      # 17
WIN2 = WIN * WIN             # 289
KC = BOX // 2                # z-basis centering (44)
KU = WIN // 2                # window-basis centering (8)
YPAD = BOX + 2 * MARGIN      # 94
VOLC = YPAD * YPAD           # 8836
ZSH = BOX // N_CORES         # 11
XPAIRW = WIN + BINV          # 28 (x-paired slab width)
SLABC = WIN * XPAIRW         # 476
PADW = -60.0                 # pad-rank W arg constant (exp -> 0)

ARG_DT = F16                 # arg matmul dtype: F16 = split hi/lo (validated
                             # 6e-4 max arg err), F32 = 4 cyc/row fallback
ARG_COLS = 128               # basis cols (122 used; pad to 256 only for f32r)
NPAR = 18 if ARG_DT == F16 else 9
NKPAR = 8 if ARG_DT == F16 else 4
CHUNK_T = 8                  # tiles per arg/exp chunk
ACT_KR_EVERY = 4             # legacy knob (unused when KR_PATTERN set)
POOL_KR_EVERY = 3            # legacy knob (unused when KR_PATTERN set)
KR_PATTERN = "dppadpa"       # kr engine per tile (cyclic): d=DVE p=Pool a=ACT
DRAIN_POOL_EVERY = 0         # every Nth drain on Pool (0=off; Pool can't PSUM)
DMA_SPLIT = 4
BIN_ALIGN = 128              # bin rank-range alignment (128 = base-0 only)
ISO_NO_KR = False            # timing isolation: skip kr ops (shared dummy kr)
ISO_NO_DRAIN = False         # timing isolation: skip drains
ISO_NO_ARG = False           # timing isolation: skip arg matmuls + exps
ISO_NO_SPLAT = False         # timing isolation: skip splat matmuls

_cache = {}


def _rotmat(quat):
    q = quat.astype(np.float64)
    q = q / np.sqrt((q * q).sum())
    w, x, y, z = q
    return np.array(
        [
            [1 - 2 * (y * y + z * z), 2 * (x * y - w * z), 2 * (x * z + w * y)],
            [2 * (x * y + w * z), 1 - 2 * (x * x + z * z), 2 * (y * z - w * x)],
            [2 * (x * z - w * y), 2 * (y * z + w * x), 1 - 2 * (x * x + y * y)],
        ],
        dtype=np.float32,
    )


# ---------------------------------------------------------------------------
# host-side prep
# ---------------------------------------------------------------------------

def _prepare(quat, offset, positions, amplitudes, variances, voxel_grid):
    quat = np.asarray(quat, np.float32)
    offset = np.asarray(offset, np.float32)
    positions = np.asarray(positions, np.float32)
    amplitudes = np.asarray(amplitudes, np.float32)
    variances = np.asarray(variances, np.float32)
    voxel_grid = np.asarray(voxel_grid, np.float32)

    rot = _rotmat(quat)
    pos = positions @ rot + offset          # [A,3], voxel units, center=idx 64
    A = positions.shape[0]

    # box origin per axis (global grid index of box cell 0)
    c0 = np.round(offset).astype(np.int64) + 64 - BOX // 2   # [3] (x,y,z order)
    c0 = np.clip(c0, 0, 128 - BOX)
    pb = pos + 64.0 - c0[None, :].astype(np.float32)          # box coords [A,3]
    px, py, pz = pb[:, 0], pb[:, 1], pb[:, 2]

    by = np.clip(np.floor(py / BINV).astype(np.int64), 0, NB - 1)
    bx = np.clip(np.floor(px / BINV).astype(np.int64), 0, NB - 1)
    bin_id = by * NB + bx
    order = np.argsort(bin_id, kind="stable")

    # per-bin atom lists (global)
    counts = np.bincount(bin_id, minlength=NB * NB)
    starts = np.concatenate([[0], np.cumsum(counts)])
    cap = np.ceil(counts / N_CORES).astype(np.int64)     # per-core padded count
    # rank ranges per bin, padded to BIN_ALIGN (PE base-partition limits)
    slots = np.ceil(cap * G / BIN_ALIGN).astype(np.int64) * BIN_ALIGN
    bin_r0 = np.concatenate([[0], np.cumsum(slots)])
    R_real = int(bin_r0[-1])
    T = (R_real + P - 1) // P
    R_pad = T * P

    # per-core rank -> atom mapping
    atom_of = np.full((N_CORES, R_pad), -1, np.int64)
    g_of = np.zeros((N_CORES, R_pad), np.int64)
    for b in range(NB * NB):
        ix = order[starts[b]:starts[b + 1]]
        for c in range(N_CORES):
            mine = ix[c::N_CORES]
            n = len(mine)
            if n == 0:
                continue
            r0 = int(bin_r0[b])
            rr = r0 + np.arange(n * G)
            atom_of[c, rr] = np.repeat(mine, G)
            g_of[c, rr] = np.tile(np.arange(G), n)

    # params
    in_maps = []
    for c in range(N_CORES):
        av = atom_of[c]
        valid = av >= 0
        a_ = np.where(valid, av, 0)
        g_ = g_of[c]
        var_r = variances[a_, g_]
        amp_r = amplitudes[a_, g_]
        sc = (-0.5 / var_r).astype(np.float32)
        lnpref = (np.log(amp_r) - 1.5 * np.log(2 * np.pi * var_r)).astype(np.float32)

        pzr = pz[a_].astype(np.float32)
        pyr = py[a_].astype(np.float32)
        pxr = px[a_].astype(np.float32)
        # bin of each rank (recompute from layout for pad safety)
        rb = np.searchsorted(bin_r0[1:], np.arange(R_pad), side="right")
        rby, rbx = rb // NB, rb % NB
        dy = pyr - (rby * BINV - MARGIN) - KU     # centered window offset
        dx = pxr - (rbx * BINV - MARGIN) - KU
        zc = pzr - KC

        pars = np.zeros((9, R_pad), np.float32)
        pars[0] = sc
        pars[1] = -2 * sc * zc
        pars[2] = sc * zc * zc + lnpref
        pars[3] = sc
        pars[4] = -2 * sc * dy
        pars[5] = sc * dy * dy
        pars[6] = sc
        pars[7] = -2 * sc * dx
        pars[8] = sc * dx * dx
        # pad ranks: W=0 (exp(PADW)), u=v=1
        pars[:, ~valid] = 0.0
        pars[2, ~valid] = PADW

        kpars = np.zeros((4, R_pad), np.float32)
        kpars[0] = pars[3]
        kpars[1] = pars[4]
        kpars[2] = pars[7]
        kpars[3] = pars[5] + pars[8]
        kpars[:, ~valid] = 0.0

        if ARG_DT == F16:
            def split16(x):
                hi = x.astype(np.float16)
                lo = (x - hi.astype(np.float32)).astype(np.float16)
                return hi, lo
            p18 = np.zeros((NPAR, R_pad), np.float16)
            for i in range(9):
                p18[2 * i], p18[2 * i + 1] = split16(pars[i])
            k8 = np.zeros((NKPAR, R_pad), np.float16)
            for i in range(4):
                k8[2 * i], k8[2 * i + 1] = split16(kpars[i])
            in_maps.append({"pars": p18, "kpars": k8})
        else:
            in_maps.append({"pars": pars, "kpars": kpars})

    bins_meta = {
        "bin_r0": bin_r0.astype(np.int64),
        "T": T,
        "R_pad": R_pad,
        "counts": counts,
        "cap": cap,
    }
    return bins_meta, in_maps, c0, voxel_grid


def _prepare_k2(c0, voxel_grid, vols):
    """vols: list of 8 partial volumes [BOX, BOX*BOX] (bf16-as-uint16 or f32)."""
    CH = BOX // N_CORES   # col chunks per z (8) -> 88 partitions = 11z x 8ch
    COLF = BOX * BOX // N_CORES   # 968
    gz0, gy0, gx0 = int(c0[2]), int(c0[1]), int(c0[0])
    in_maps = []
    for c in range(N_CORES):
        # vstack [88, 8*968]: partition q = a*8 + ch; free = p*968 + f
        vst = np.zeros((BOX, N_CORES * COLF), dtype=vols[0].dtype)
        for p in range(N_CORES):
            sl = vols[p][ZSH * c:ZSH * (c + 1)]          # [11, 7744]
            s3 = sl.reshape(ZSH, N_CORES, COLF)          # [11, 8ch, 968]
            vst[:, p * COLF:(p + 1) * COLF] = s3.reshape(BOX, COLF)
        gsl = voxel_grid[gz0 + ZSH * c: gz0 + ZSH * (c + 1),
                         gy0:gy0 + BOX, gx0:gx0 + BOX]   # [11, 88, 88] f32
        g2 = gsl.reshape(ZSH, N_CORES, COLF).reshape(BOX, COLF)
        import ml_dtypes
        g2b = np.ascontiguousarray(g2, np.float32).astype(ml_dtypes.bfloat16)
        in_maps.append({"vstack": vst, "g2": g2b})
    return in_maps


# ---------------------------------------------------------------------------
# device program k1: splat partial volume
# ---------------------------------------------------------------------------

def _build_k1(bins_meta, loop_reps=0):
    T = bins_meta["T"]
    R_pad = bins_meta["R_pad"]
    bin_r0 = bins_meta["bin_r0"]

    nc = bacc.Bacc("TRN2", target_bir_lowering=False, debug=False,
                   num_devices=N_CORES)
    pars_d = nc.dram_tensor("pars", [NPAR, R_pad], ARG_DT, kind="ExternalInput")
    kpars_d = nc.dram_tensor("kpars", [NKPAR, R_pad], ARG_DT,
                             kind="ExternalInput")
    # full padded vol (contiguous DMA; host trims the margins)
    vol_d = nc.dram_tensor("vol", [BOX, VOLC], BF16, kind="ExternalOutput")

    # basis constants
    bas9 = np.zeros((9, ARG_COLS), np.float32)
    zc = np.arange(BOX, dtype=np.float32) - KC
    bas9[0, :BOX] = zc * zc
    bas9[1, :BOX] = zc
    bas9[2, :BOX] = 1.0
    wc = np.arange(WIN, dtype=np.float32) - KU
    bas9[3, BOX:BOX + WIN] = wc * wc
    bas9[4, BOX:BOX + WIN] = wc
    bas9[5, BOX:BOX + WIN] = 1.0
    bas9[6, BOX + WIN:BOX + 2 * WIN] = wc * wc
    bas9[7, BOX + WIN:BOX + 2 * WIN] = wc
    bas9[8, BOX + WIN:BOX + 2 * WIN] = 1.0
    # kr is x-major: col = x*WIN + w
    kb4 = np.zeros((4, WIN2), np.float32)
    ww = np.tile(wc, WIN)
    xx = np.repeat(wc, WIN)
    kb4[0] = ww * ww + xx * xx
    kb4[1] = ww
    kb4[2] = xx
    kb4[3] = 1.0
    if ARG_DT == F16:
        bas_np = np.repeat(bas9, 2, axis=0).astype(np.float16)
        kb_np = np.repeat(kb4, 2, axis=0).astype(np.float16)
    else:
        bas_np, kb_np = bas9, kb4
    bas_c = nc.inline_tensor(bas_np, name="bas_c")
    kb_c = nc.inline_tensor(kb_np, name="kb_c")

    UOFF = BOX            # u cols offset within tile block
    VOFF = BOX + WIN
    TB = BOX + 2 * WIN    # 122 used cols per tile block
    NCHUNK = (T + CHUNK_T - 1) // CHUNK_T

    with tile.TileContext(nc) as tc:
        with tc.tile_pool(name="keep", bufs=1) as keep:
            bas = keep.tile([NPAR, ARG_COLS], ARG_DT)
            nc.sync.dma_start(bas[:], bas_c[:, :])
            kb = keep.tile([NKPAR, WIN2], ARG_DT)
            nc.sync.dma_start(kb[:], kb_c[:, :])
            pars = keep.tile([NPAR, R_pad], ARG_DT)
            kpars = keep.tile([NKPAR, R_pad], ARG_DT)
            wuv = keep.tile([P, T * P], BF16)
            vol = keep.tile([BOX, VOLC], BF16)

            # one-time zeros in DRAM; per-rep vol clear is then DMA-only
            zdp = tc.tile_pool(name="zd", bufs=1, space="DRAM")
            zd = zdp.__enter__()
            zer_d = zd.tile([BOX, VOLC], BF16)
            third = VOLC // 4
            nc.vector.memset(vol[:, :third], 0.0)
            nc.vector.memset(vol[:, third:2 * third], 0.0)
            nc.gpsimd.memset(vol[:, 2 * third:3 * third], 0.0)
            nc.gpsimd.memset(vol[:, 3 * third:], 0.0)
            cwz = VOLC // DMA_SPLIT
            for s in range(DMA_SPLIT):
                nc.sync.dma_start(zer_d[:, s * cwz:(s + 1) * cwz],
                                  vol[:, s * cwz:(s + 1) * cwz])

            def body():
                cw = R_pad // DMA_SPLIT
                for s in range(DMA_SPLIT):
                    nc.sync.dma_start(pars[:, s * cw:(s + 1) * cw],
                                      pars_d[:, s * cw:(s + 1) * cw])
                    nc.sync.dma_start(kpars[:, s * cw:(s + 1) * cw],
                                      kpars_d[:, s * cw:(s + 1) * cw])
                _k1_body(nc, tc, bins_meta, bas, kb, pars, kpars, wuv, vol,
                         zer_d)
                # full padded vol -> DRAM (contiguous), split along cols
                cw = VOLC // DMA_SPLIT
                for s in range(DMA_SPLIT):
                    nc.sync.dma_start(vol_d[:, s * cw:(s + 1) * cw],
                                      vol[:, s * cw:(s + 1) * cw])

            if loop_reps:
                with tc.For_i(0, loop_reps, 1):
                    body()
            else:
                body()
            zdp.__exit__(None, None, None)

    nc.compile()
    return nc


def _k1_body(nc, tc, bins_meta, bas, kb, pars, kpars, wuv, vol, zer_d):
    T = bins_meta["T"]
    bin_r0 = bins_meta["bin_r0"]
    NCHUNK = (T + CHUNK_T - 1) // CHUNK_T
    UOFF = BOX
    VOFF = BOX + WIN
    TB = BOX + 2 * WIN

    # vol clear via DMA from the DRAM zeros buffer (no engine time)
    cwz = VOLC // DMA_SPLIT
    for s in range(DMA_SPLIT):
        nc.sync.dma_start(vol[:, s * cwz:(s + 1) * cwz],
                          zer_d[:, s * cwz:(s + 1) * cwz])

    with tc.tile_pool(name="work", bufs=8) as wk, \
         tc.tile_pool(name="argp", bufs=2, space="PSUM") as argp, \
         tc.tile_pool(name="slabp", bufs=2, space="PSUM") as slabp, \
         tc.tile_pool(name="kkp", bufs=2, space="PSUM") as kkp:

        def emit_chunk(cc):
            if cc >= NCHUNK or ISO_NO_ARG:
                return
            t0 = cc * CHUNK_T
            n = min(CHUNK_T, T - t0)
            ac = argp.tile([P, CHUNK_T * ARG_COLS], F32, tag="ac", bufs=2)
            for j in range(n):
                t = t0 + j
                nc.tensor.matmul(out=ac[:, ts(j, ARG_COLS)],
                                 lhsT=pars[:, ts(t, P)], rhs=bas[:, :],
                                 start=True, stop=True)
            ac3 = ac[:].rearrange("p (j c) -> p j c", c=ARG_COLS)
            wv3 = wuv[:, t0 * P:(t0 + n) * P].rearrange(
                "p (j c) -> p j c", c=P)
            nc.scalar.activation(wv3[:, :, :TB], ac3[:, :n, :TB], AF.Exp)

        # software pipeline: keep one arg chunk ahead of splat tiles
        emit_chunk(0)
        emit_chunk(1)

        kr_shared = None
        if ISO_NO_KR:
            kr_shared = wk.tile([P, WIN2], BF16, tag="krsh", bufs=1)
            nc.vector.memset(kr_shared[:], 0.1)

        # build slab-group schedule: 32 groups of x-paired bins
        # group (by, bp): bins b0=(by, 2bp), b1=(by, 2bp+1)
        kr_tiles = {}     # t -> kr tile handle

        emitted = [2]     # chunks 0,1 pre-emitted

        def get_kr(t):
            if t in kr_tiles:
                return kr_tiles[t]
            # pipeline arg chunks (order-safe: lookahead past max tile seen)
            while emitted[0] <= t // CHUNK_T + 2:
                emit_chunk(emitted[0])
                emitted[0] += 1
            if ISO_NO_KR:
                kr_tiles[t] = kr_shared
                return kr_shared
            eng_c = KR_PATTERN[t % len(KR_PATTERN)]
            use_act = eng_c == "a"
            kr = wk.tile([P, WIN2], BF16, tag="kr", bufs=24)
            if use_act:
                pk = kkp.tile([P, WIN2], F32, tag="pk", bufs=2)
                nc.tensor.matmul(out=pk[:], lhsT=kpars[:, ts(t, P)],
                                 rhs=kb[:, :], start=True, stop=True)
                nc.scalar.activation(kr[:], pk[:], AF.Exp)
            else:
                eng = nc.gpsimd if eng_c == "p" else nc.vector
                # x-major: kr[p, x, w] = v[p,x] * u[p,w]
                kr3 = kr[:].rearrange("p (x w) -> p x w", w=WIN)
                u = wuv[:, t * P + UOFF:t * P + UOFF + WIN]
                v = wuv[:, t * P + VOFF:t * P + VOFF + WIN]
                eng.tensor_tensor(
                    out=kr3[:],
                    in0=v.unsqueeze(2).to_broadcast([P, WIN, WIN]),
                    in1=u.unsqueeze(1).to_broadcast([P, WIN, WIN]),
                    op=OP.mult)
            kr_tiles[t] = kr
            return kr

        drain_i = 0
        for by in range(NB):
            # interleaved x-pair order: consecutive drains touch disjoint
            # vol regions, breaking the margin-overlap WAR chain
            for bp in (0, 2, 1, 3):
                b0 = by * NB + 2 * bp
                b1 = b0 + 1
                r0a, r1a = int(bin_r0[b0]), int(bin_r0[b0 + 1])
                r0b, r1b = int(bin_r0[b1]), int(bin_r0[b1 + 1])
                n_a = r1a - r0a
                n_b = r1b - r0b
                if n_a == 0 and n_b == 0:
                    continue
                # slab is x-major: col = x*WIN + w; x in [0, XPAIRW)
                # bin A covers cols [0, WIN2); bin B covers [BINV*WIN, SLABC)
                BOFF = BINV * WIN        # 187
                OVW = (WIN - BINV) * WIN  # 102 (x-overlap cols within a bin)
                slab = slabp.tile([BOX, SLABC], F32, tag="slab", bufs=2)

                def frags(r0, r1):
                    # 64-aligned (t, p0, p1) segments (base partition 0/64)
                    out = []
                    r = r0
                    while r < r1:
                        t = r // P
                        p0 = r - t * P
                        p1 = min(r1 - t * P, P)
                        out.append((t, p0, p1))
                        r = t * P + p1
                    return out

                # Collect matmuls as (tile, p0, p1, out_lo, out_hi, kr_lo,
                # kr_hi); PSUM pending-zero handles first-write-vs-accumulate
                # per element, but each mm must touch uniformly-pending bytes,
                # so the first touch of B's exclusive region is split out.
                # HW has_written is per-element: a matmul may mix first-write
                # and accumulate bytes (skip the interp-only group check).
                mms = []
                fa = frags(r0a, r1a)
                fb = frags(r0b, r1b)
                for i, (t, p0, p1) in enumerate(fa):
                    mms.append((t, p0, p1, 0, WIN2, 0, WIN2))
                for i, (t, p0, p1) in enumerate(fb):
                    mms.append((t, p0, p1, BOFF, SLABC, 0, WIN2))
                for i, (t, p0, p1, olo, ohi, klo, khi) in enumerate(mms):
                    kr = get_kr(t)
                    if ISO_NO_SPLAT:
                        continue
                    lhsT = wuv[p0:p1, ts(t, P)][:, :BOX]
                    nc.tensor.matmul(out=slab[:, olo:ohi],
                                     lhsT=lhsT, rhs=kr[p0:p1, klo:khi],
                                     start=(i == 0), stop=(i == len(mms) - 1),
                                     skip_group_check=True)
                pass

                # drain slab -> vol window (skip regions no bin wrote)
                y0 = by * BINV
                x0 = 2 * bp * BINV
                vol3 = vol[:].rearrange("p (y x) -> p y x", x=YPAD)
                if n_a > 0 and n_b > 0:
                    xlo, xhi = 0, XPAIRW
                elif n_a > 0:
                    xlo, xhi = 0, WIN
                else:
                    xlo, xhi = BINV, XPAIRW
                if ISO_NO_DRAIN:
                    continue
                dst = vol3[:, y0:y0 + WIN, x0 + xlo:x0 + xhi]
                # slab is x-major [p, x, w]; drain wants [p, w(y), x]
                slabwx = slab[:].rearrange("p (x w) -> p w x", w=WIN)
                use_pool = DRAIN_POOL_EVERY and (drain_i % DRAIN_POOL_EVERY
                                                 == DRAIN_POOL_EVERY - 1)
                eng = nc.gpsimd if use_pool else nc.vector
                eng.tensor_tensor(out=dst, in0=dst,
                                  in1=slabwx[:, :, xlo:xhi], op=OP.add)
                drain_i += 1
            # stale kr handles may alias recycled pool buffers across rows
            kr_tiles.clear()


# ---------------------------------------------------------------------------
# device program k2: sum partial z-shards + reduce
# ---------------------------------------------------------------------------

def _build_k2(loop_reps=0):
    COLF = BOX * BOX // N_CORES   # 968
    nc = bacc.Bacc("TRN2", target_bir_lowering=False, debug=False,
                   num_devices=N_CORES)
    vst_d = nc.dram_tensor("vstack", [BOX, N_CORES * COLF], BF16,
                           kind="ExternalInput")
    g2_d = nc.dram_tensor("g2", [BOX, COLF], BF16, kind="ExternalInput")
    out_d = nc.dram_tensor("res2", [2, 1], F32, kind="ExternalOutput")
    ones_np = np.ones((BOX, 1), np.float32)
    ones_c = nc.inline_tensor(ones_np, name="ones_c")

    with tile.TileContext(nc) as tc:
        with tc.tile_pool(name="p", bufs=1) as p, \
             tc.tile_pool(name="ps", bufs=1, space="PSUM") as ps:
            vst = p.tile([BOX, N_CORES * COLF], BF16)
            g2 = p.tile([BOX, COLF], BF16)
            ones = p.tile([BOX, 1], F32)
            nc.sync.dma_start(ones[:], ones_c[:, :])

            def body():
                for s in range(2 * DMA_SPLIT):
                    cw = N_CORES * COLF // (2 * DMA_SPLIT)
                    nc.sync.dma_start(vst[:, s * cw:(s + 1) * cw],
                                      vst_d[:, s * cw:(s + 1) * cw])
                nc.sync.dma_start(g2[:], g2_d[:, :])
                v3 = vst[:].rearrange("p (s f) -> p s f", f=COLF)
                # pairwise tree sum into source slot 0 region (bf16)
                for step in (1, 2, 4):
                    for s0 in range(0, N_CORES, 2 * step):
                        nc.vector.tensor_tensor(
                            out=v3[:, s0, :], in0=v3[:, s0, :],
                            in1=v3[:, s0 + step, :], op=OP.add)
                vsum = v3[:, 0, :]
                prod = p.tile([BOX, COLF], BF16, tag="prod")
                parts = p.tile([BOX, 2], F32, tag="parts")
                nc.vector.tensor_tensor(out=prod[:], in0=vsum, in1=vsum,
                                        op=OP.mult)
                nc.vector.tensor_reduce(out=parts[:, 0:1], in_=prod[:],
                                        axis=mybir.AxisListType.X, op=OP.add)
                nc.vector.tensor_tensor(out=prod[:], in0=vsum, in1=g2[:],
                                        op=OP.mult)
                nc.vector.tensor_reduce(out=parts[:, 1:2], in_=prod[:],
                                        axis=mybir.AxisListType.X, op=OP.add)
                red = ps.tile([2, 1], F32, tag="red")
                nc.tensor.matmul(out=red[:], lhsT=parts[:, :], rhs=ones[:, :],
                                 start=True, stop=True)
                red_sb = p.tile([2, 1], F32, tag="red_sb")
                nc.scalar.copy(red_sb[:], red[:])
                return red_sb

            if loop_reps:
                with tc.For_i(0, loop_reps, 1):
                    red_sb = body()
            else:
                red_sb = body()
            nc.sync.dma_start(out_d[:, :], red_sb[:])

    nc.compile()
    return nc


# ---------------------------------------------------------------------------
# entry
# ---------------------------------------------------------------------------

def kernel(quat, offset, positions, amplitudes, variances, voxel_grid):
    bins_meta, in_maps, c0, g = _prepare(
        quat, offset, positions, amplitudes, variances, voxel_grid)
    key = ("k1", bins_meta["T"], tuple(bins_meta["bin_r0"].tolist()))
    if key not in _cache:
        _cache[key] = _build_k1(bins_meta)
    nc1 = _cache[key]
    res1 = run_bass_kernel_spmd(nc1, in_maps, core_ids=list(range(N_CORES)))
    vols = []
    for c in range(N_CORES):
        vp = res1.results[c]["vol"].reshape(BOX, YPAD, YPAD)
        vols.append(np.ascontiguousarray(
            vp[:, MARGIN:MARGIN + BOX, MARGIN:MARGIN + BOX]).reshape(BOX, -1))

    in2 = _prepare_k2(c0, g, vols)
    if "k2" not in _cache:
        _cache["k2"] = _build_k2()
    nc2 = _cache["k2"]
    res2 = run_bass_kernel_spmd(nc2, in2, core_ids=list(range(N_CORES)))

    ssq = dot = 0.0
    for c in range(N_CORES):
        r = res2.results[c]["res2"]
        ssq += float(r[0, 0])
        dot += float(r[1, 0])
    gssq = float((g.astype(np.float64) ** 2).sum())
    corr = dot / math.sqrt(ssq * gssq)
    return np.float32(1.0 - corr)



# revision 43
# speedup vs baseline: 1.2345x; 1.2345x over previous
"""Trainium2 Bass kernel for ModelToVolumeAligner — v3 (2D-binned, collective-free).

Strategy:
  - Host: rotate positions, clip to an 88^3 box centered on the cloud,
    2D-bin atoms by (y,x) into 8x8 bins of 11 voxels, shard atoms per bin
    round-robin across 8 cores (uniform compile-time layout, 64-aligned
    per-bin rank ranges), precompute per-rank quadratic exp-arg params.
  - Device k1 (per core): per 128-rank tile, ONE small matmul (contract
    18 f16 hi/lo rows) against a fixed basis computes the exp-arguments
    for the z-profile W (88 cols) and the y/x window profiles u,v
    (13 cols each, MARGIN=1); batched ACT Exp over 8-tile chunks -> bf16.
    kr = u (x) v via broadcast TT on Pool or DVE (greedy load balance).
    Splat: matmul contracting ranks: slab[z, 13x13 window] += W^T @ kr,
    accumulated in PSUM across x-paired bins; slabs drained into a bf16
    vol [88, 90x90] either directly on DVE or via ACT copy + Pool add
    (greedy balance). Interior [88,88,88] DMA'd out in y-chunks as soon
    as each y-row of bins completes (overlaps compute).
  - Host: restack 8 partial volumes into per-core z-shards.
  - Device k2 (per core): column-chunked pipeline; pairwise-tree add of
    the 8 partial z-shards on alternating DVE/Pool, then ssq/dot
    reductions vs the clipped voxel grid -> [2*CH] scalars.
  - Host: combine scalars + host gssq -> 1 - dot/sqrt(ssq*gssq).
    (The v/v.sum() normalization cancels in the correlation.)
  - No collectives: ReduceScatter in this environment costs ~1ms fixed.
"""

import math
import numpy as np

import concourse.bass as bass
import concourse.mybir as mybir
import concourse.tile as tile
from concourse import bacc
from concourse.bass import ts
from concourse.bass_utils import run_bass_kernel_spmd

F32 = mybir.dt.float32
F16 = mybir.dt.float16
BF16 = mybir.dt.bfloat16
AF = mybir.ActivationFunctionType
OP = mybir.AluOpType

N_CORES = 8
P = 128
G = 5
BOX = 88            # clipped cube side (voxels)
NB = 8              # bins per axis
BINV = BOX // NB    # 11 voxels per bin
MARGIN = 1
WIN = BINV + 2 * MARGIN      # 13
WIN2 = WIN * WIN             # 169
KC = BOX // 2                # z-basis centering (44)
KU = WIN // 2                # window-basis centering (6)
YPAD = BOX + 2 * MARGIN      # 90
XSTRIDE = 112                # padded x-row stride (fits 2-pair drain views)
VOLC = YPAD * XSTRIDE        # 10080
ZSH = BOX // N_CORES         # 11
XPAIRW = WIN + BINV          # 24 (x-paired slab width)
SLABC = WIN * XPAIRW         # 312
PADW = -60.0                 # pad-rank W arg constant (exp -> 0)

NPAR = 18                    # f16 hi/lo split of 9 quadratic params
ARG_COLS = 128               # basis tile width (114 used)
TB = BOX + 2 * WIN           # 114 used cols per tile block
UOFF = BOX                   # u cols offset within tile block
VOFF = BOX + WIN
CHUNK_T = 8                  # tiles per arg/exp chunk
ARGP_BUFS = 2                # in-flight arg chunks (PSUM banks)
LOOKAHEAD = 2                # arg chunks emitted ahead of kr consumption
KR_BATCH = 4                 # tiles per kr TT instruction (1 = per-tile 2D)
DRAIN_BATCH = True           # drain both couple pairs in one 3-free-dim TT
OUT_CHUNKS = True            # stream interior vol out per y-row of bins
ROWS_EMIT = NB               # debug: emit only the first k bin-rows
DMA_SPLIT = 4
BIN_ALIGN = 64               # bin rank-range alignment (PE base partition 0/64)
K2_CH = 2                    # k2 column chunks

# engine cost constants (ns) for greedy balancing, calibrated against
# timeline-sim slice durations of this exact build:
#   Pool broadcast-TT ~2.0 ns/el + 95 launch; DVE TT 1.042 ns/el + inits;
#   ACT copy 0.833 ns/el + 186; Pool SBUF add 1.39 ns/el + 95.
C_KR_POOL_FIX, C_KR_POOL_EL = 95.0, 1.98 * WIN2
C_KR_DVE_FIX, C_KR_DVE_EL = 60.0, 1.042 * WIN2
C_DR_DVE = 125.0             # + cols*1.042 (PSUM access latency on DVE)
C_DR_ACT = 186.0             # + cols*0.833 (copy PSUM->SBUF)
C_DR_POOL = 95.0             # + cols*1.98 (SBUF add)
C_ACT_CHUNK_FIX = 130.0

_cache = {}


def _rotmat(quat):
    q = quat.astype(np.float64)
    q = q / np.sqrt((q * q).sum())
    w, x, y, z = q
    return np.array(
        [
            [1 - 2 * (y * y + z * z), 2 * (x * y - w * z), 2 * (x * z + w * y)],
            [2 * (x * y + w * z), 1 - 2 * (x * x + z * z), 2 * (y * z - w * x)],
            [2 * (x * z - w * y), 2 * (y * z + w * x), 1 - 2 * (x * x + y * y)],
        ],
        dtype=np.float32,
    )


# ---------------------------------------------------------------------------
# host-side prep
# ---------------------------------------------------------------------------

def _prepare(quat, offset, positions, amplitudes, variances, voxel_grid):
    quat = np.asarray(quat, np.float32)
    offset = np.asarray(offset, np.float32)
    positions = np.asarray(positions, np.float32)
    amplitudes = np.asarray(amplitudes, np.float32)
    variances = np.asarray(variances, np.float32)
    voxel_grid = np.asarray(voxel_grid, np.float32)

    rot = _rotmat(quat)
    pos = positions @ rot + offset          # [A,3], voxel units, center=idx 64
    A = positions.shape[0]

    # box origin per axis (global grid index of box cell 0)
    c0 = np.round(offset).astype(np.int64) + 64 - BOX // 2   # [3] (x,y,z order)
    c0 = np.clip(c0, 0, 128 - BOX)
    pb = pos + 64.0 - c0[None, :].astype(np.float32)          # box coords [A,3]
    px, py, pz = pb[:, 0], pb[:, 1], pb[:, 2]

    by = np.clip(np.floor(py / BINV).astype(np.int64), 0, NB - 1)
    bx = np.clip(np.floor(px / BINV).astype(np.int64), 0, NB - 1)
    bin_id = by * NB + bx
    order = np.argsort(bin_id, kind="stable")

    # per-bin atom lists (global)
    counts = np.bincount(bin_id, minlength=NB * NB)
    starts = np.concatenate([[0], np.cumsum(counts)])
    cap = np.ceil(counts / N_CORES).astype(np.int64)     # per-core padded count
    # rank ranges per bin, padded to BIN_ALIGN units
    units = np.ceil(cap * G / BIN_ALIGN).astype(np.int64)   # 64-rank units

    # HW constraint: two in-flight matmuls on the same PSUM bank with
    # DISJOINT partition ranges execute concurrently (per-subarray
    # LDWEIGHTS pull-ahead) and fault.  Within each accumulation group,
    # consecutive matmul frags must therefore INTERSECT in partitions.
    # Per pair, search emit orders / solo-split / padding bumps.
    def fragseq(phi, n):
        # frag partition intervals for n 64-units starting at 64-phase phi
        out = []
        r = phi
        end = phi + n
        while r < end:
            t = r // 2
            p0 = (r - t * 2) * 64
            p1 = min(end - t * 2, 2) * 64
            out.append((p0, p1))
            r = t * 2 + min(end - t * 2, 2)
        return out

    def seq_ok(seq):
        return all(not (a[1] <= b[0] or b[1] <= a[0])
                   for a, b in zip(seq, seq[1:]))

    pair_plan = []    # per pair: list of groups, each = list of local bin idx
    units = units.copy()
    phi = 0
    for pr in range(NB * NB // 2):
        b0 = 2 * pr
        for bump in range(4):
            a, b = int(units[b0]), int(units[b0 + 1])
            f0 = fragseq(phi, a)
            f1 = fragseq(phi + a, b)
            found = None
            if seq_ok(f0 + f1):
                found = [[0, 1]]
            elif seq_ok(f1 + f0):
                found = [[1, 0]]
            elif seq_ok(f0) and seq_ok(f1):
                found = [[0], [1]]
            if found is not None:
                pair_plan.append(found)
                break
            # bump the smaller bin by one unit and retry
            if a <= b:
                units[b0] += 1
            else:
                units[b0 + 1] += 1
        else:
            raise RuntimeError("layout fixer failed")
        phi += int(units[b0] + units[b0 + 1])

    slots = units * BIN_ALIGN
    bin_r0 = np.concatenate([[0], np.cumsum(slots)])
    R_real = int(bin_r0[-1])
    T = (R_real + P - 1) // P
    R_pad = T * P

    # per-core rank -> atom mapping
    atom_of = np.full((N_CORES, R_pad), -1, np.int64)
    g_of = np.zeros((N_CORES, R_pad), np.int64)
    for b in range(NB * NB):
        ix = order[starts[b]:starts[b + 1]]
        for c in range(N_CORES):
            mine = ix[c::N_CORES]
            n = len(mine)
            if n == 0:
                continue
            r0 = int(bin_r0[b])
            rr = r0 + np.arange(n * G)
            atom_of[c, rr] = np.repeat(mine, G)
            g_of[c, rr] = np.tile(np.arange(G), n)

    # params
    in_maps = []
    for c in range(N_CORES):
        av = atom_of[c]
        valid = av >= 0
        a_ = np.where(valid, av, 0)
        g_ = g_of[c]
        var_r = variances[a_, g_]
        amp_r = amplitudes[a_, g_]
        sc = (-0.5 / var_r).astype(np.float32)
        lnpref = (np.log(amp_r) - 1.5 * np.log(2 * np.pi * var_r)).astype(np.float32)

        pzr = pz[a_].astype(np.float32)
        pyr = py[a_].astype(np.float32)
        pxr = px[a_].astype(np.float32)
        # bin of each rank (recompute from layout for pad safety)
        rb = np.searchsorted(bin_r0[1:], np.arange(R_pad), side="right")
        rb = np.minimum(rb, NB * NB - 1)
        rby, rbx = rb // NB, rb % NB
        dy = pyr - (rby * BINV - MARGIN) - KU     # centered window offset
        dx = pxr - (rbx * BINV - MARGIN) - KU
        zc = pzr - KC

        pars = np.zeros((9, R_pad), np.float32)
        pars[0] = sc
        pars[1] = -2 * sc * zc
        pars[2] = sc * zc * zc + lnpref
        pars[3] = sc
        pars[4] = -2 * sc * dy
        pars[5] = sc * dy * dy
        pars[6] = sc
        pars[7] = -2 * sc * dx
        pars[8] = sc * dx * dx
        # pad ranks: W=0 (exp(PADW)), u=v=1
        pars[:, ~valid] = 0.0
        pars[2, ~valid] = PADW

        def split16(x):
            hi = x.astype(np.float16)
            lo = (x - hi.astype(np.float32)).astype(np.float16)
            return hi, lo
        p18 = np.zeros((NPAR, R_pad), np.float16)
        for i in range(9):
            p18[2 * i], p18[2 * i + 1] = split16(pars[i])
        in_maps.append({"pars": p18})

    bins_meta = {
        "bin_r0": bin_r0.astype(np.int64),
        "T": T,
        "R_pad": R_pad,
        "counts": counts,
        "cap": cap,
        "pair_plan": pair_plan,
    }
    return bins_meta, in_maps, c0, voxel_grid


K2_COLF = BOX * BOX // N_CORES            # 968
K2_CWS = (208, K2_COLF - 208)             # chunk widths: Pool small, DVE big


def _prepare_k2(c0, voxel_grid, vols):
    """vols: list of 8 partial interior volumes [BOX, BOX*BOX] bf16."""
    COLF = K2_COLF
    gz0, gy0, gx0 = int(c0[2]), int(c0[1]), int(c0[0])
    in_maps = []
    for c in range(N_CORES):
        # vstack chunk-major [88, (chunk, 8, cw)]: chunk DMAs are contiguous
        vst = np.zeros((BOX, N_CORES * COLF), dtype=vols[0].dtype)
        for p in range(N_CORES):
            sl = vols[p][ZSH * c:ZSH * (c + 1)]          # [11, 7744]
            s3 = sl.reshape(ZSH, N_CORES, COLF)          # [11, 8ch, 968]
            vst[:, p * COLF:(p + 1) * COLF] = s3.reshape(BOX, COLF)
        v4 = vst.reshape(BOX, N_CORES, COLF)
        parts = []
        f0 = 0
        for cw in K2_CWS:
            parts.append(v4[:, :, f0:f0 + cw].reshape(BOX, -1))
            f0 += cw
        vst = np.concatenate(parts, axis=1)
        gsl = voxel_grid[gz0 + ZSH * c: gz0 + ZSH * (c + 1),
                         gy0:gy0 + BOX, gx0:gx0 + BOX]   # [11, 88, 88] f32
        g2 = gsl.reshape(ZSH, N_CORES, COLF).reshape(BOX, COLF)
        import ml_dtypes
        g2b = np.ascontiguousarray(g2, np.float32).astype(ml_dtypes.bfloat16)
        in_maps.append({"vstack": vst, "g2": g2b})
    return in_maps


# ---------------------------------------------------------------------------
# device program k1: splat partial volume
# ---------------------------------------------------------------------------

def _build_k1(bins_meta, loop_reps=0, unroll=1):
    T = bins_meta["T"]
    R_pad = bins_meta["R_pad"]

    nc = bacc.Bacc("TRN2", target_bir_lowering=False, debug=False,
                   num_devices=N_CORES)
    pars_d = nc.dram_tensor("pars", [NPAR, R_pad], F16, kind="ExternalInput")
    # interior volume only (host restacks directly)
    vol_d = nc.dram_tensor("vol", [BOX, BOX * BOX], BF16, kind="ExternalOutput")

    # basis constants
    bas9 = np.zeros((9, ARG_COLS), np.float32)
    zc = np.arange(BOX, dtype=np.float32) - KC
    bas9[0, :BOX] = zc * zc
    bas9[1, :BOX] = zc
    bas9[2, :BOX] = 1.0
    wc = np.arange(WIN, dtype=np.float32) - KU
    bas9[3, UOFF:UOFF + WIN] = wc * wc
    bas9[4, UOFF:UOFF + WIN] = wc
    bas9[5, UOFF:UOFF + WIN] = 1.0
    bas9[6, VOFF:VOFF + WIN] = wc * wc
    bas9[7, VOFF:VOFF + WIN] = wc
    bas9[8, VOFF:VOFF + WIN] = 1.0
    bas_np = np.repeat(bas9, 2, axis=0).astype(np.float16)
    bas_c = nc.inline_tensor(bas_np, name="bas_c")

    with tile.TileContext(nc) as tc:
        with tc.tile_pool(name="keep", bufs=1) as keep:
            bas = keep.tile([NPAR, ARG_COLS], F16)
            nc.sync.dma_start(bas[:], bas_c[:, :])
            pars = keep.tile([NPAR, R_pad], F16)
            wuv = keep.tile([P, T * P], BF16)
            vol = keep.tile([BOX, VOLC], BF16)

            # one-time zeros in DRAM; per-rep vol clear is then DMA-only
            zdp = tc.tile_pool(name="zd", bufs=1, space="DRAM")
            zd = zdp.__enter__()
            zer_d = zd.tile([BOX, VOLC], BF16)
            quart = VOLC // 4
            nc.vector.memset(vol[:, :quart], 0.0)
            nc.vector.memset(vol[:, quart:2 * quart], 0.0)
            nc.gpsimd.memset(vol[:, 2 * quart:3 * quart], 0.0)
            nc.gpsimd.memset(vol[:, 3 * quart:], 0.0)
            cwz = VOLC // DMA_SPLIT
            for s in range(DMA_SPLIT):
                nc.sync.dma_start(zer_d[:, s * cwz:(s + 1) * cwz],
                                  vol[:, s * cwz:(s + 1) * cwz])

            def body():
                cw = R_pad // DMA_SPLIT
                for s in range(DMA_SPLIT):
                    nc.sync.dma_start(pars[:, s * cw:(s + 1) * cw],
                                      pars_d[:, s * cw:(s + 1) * cw])
                _k1_body(nc, tc, bins_meta, bas, pars, wuv, vol, zer_d, vol_d)

            if loop_reps:
                with tc.For_i(0, loop_reps, 1):
                    body()
            else:
                for _ in range(unroll):
                    body()
            zdp.__exit__(None, None, None)

    nc.compile()
    return nc


def _k1_body(nc, tc, bins_meta, bas, pars, wuv, vol, zer_d, vol_d):
    T = bins_meta["T"]
    bin_r0 = bins_meta["bin_r0"]
    NCHUNK = (T + CHUNK_T - 1) // CHUNK_T

    # vol clear via DMA from the DRAM zeros buffer (no engine time)
    cwz = VOLC // DMA_SPLIT
    for s in range(DMA_SPLIT):
        nc.sync.dma_start(vol[:, s * cwz:(s + 1) * cwz],
                          zer_d[:, s * cwz:(s + 1) * cwz])

    # greedy engine load tracker (ns); ACT preloaded with fixed chunk exps
    load = {"dve": 0.0, "pool": 0.0,
            "act": NCHUNK * (CHUNK_T * TB * 1.03 + C_ACT_CHUNK_FIX)}

    with tc.tile_pool(name="work", bufs=8) as wk, \
         tc.tile_pool(name="argp", bufs=ARGP_BUFS, space="PSUM") as argp, \
         tc.tile_pool(name="slabp", bufs=2, space="PSUM") as slabp:

        def emit_chunk(cc):
            if cc >= NCHUNK:
                return
            t0 = cc * CHUNK_T
            n = min(CHUNK_T, T - t0)
            ac = argp.tile([P, CHUNK_T * ARG_COLS], F32, tag="ac",
                           bufs=ARGP_BUFS)
            for j in range(n):
                t = t0 + j
                nc.tensor.matmul(out=ac[:, j * ARG_COLS:j * ARG_COLS + TB],
                                 lhsT=pars[:, ts(t, P)], rhs=bas[:, :TB],
                                 start=True, stop=True)
            ac3 = ac[:].rearrange("p (j c) -> p j c", c=ARG_COLS)
            wv3 = wuv[:, t0 * P:(t0 + n) * P].rearrange(
                "p (j c) -> p j c", c=P)
            nc.scalar.activation(wv3[:, :, :TB], ac3[:, :n, :TB], AF.Exp)

        # software pipeline: keep arg chunks ahead of splat tiles
        for cc in range(LOOKAHEAD):
            emit_chunk(cc)

        kr_groups = {}    # g -> kr4 tile handle (tiles 4g..4g+3 batched)
        emitted = [LOOKAHEAD]

        def get_kr(t):
            g = t // KR_BATCH
            if g not in kr_groups:
                t0 = g * KR_BATCH
                nb = min(KR_BATCH, T - t0)
                # pipeline arg chunks past the whole batch
                while emitted[0] <= (t0 + nb - 1) // CHUNK_T + LOOKAHEAD:
                    emit_chunk(emitted[0])
                    emitted[0] += 1
                c_pool = C_KR_POOL_FIX + nb * C_KR_POOL_EL
                c_dve = C_KR_DVE_FIX + nb * C_KR_DVE_EL
                if load["pool"] + c_pool <= load["dve"] + c_dve:
                    eng = nc.gpsimd
                    load["pool"] += c_pool
                else:
                    eng = nc.vector
                    load["dve"] += c_dve
                kr4 = wk.tile([P, KR_BATCH * WIN2], BF16, tag="kr", bufs=8)
                if KR_BATCH == 1:
                    kr3 = kr4[:].rearrange("p (x w) -> p x w", w=WIN)
                    u = wuv[:, t0 * P + UOFF:t0 * P + UOFF + WIN]
                    v = wuv[:, t0 * P + VOFF:t0 * P + VOFF + WIN]
                    eng.tensor_tensor(
                        out=kr3[:],
                        in0=v.unsqueeze(2).to_broadcast([P, WIN, WIN]),
                        in1=u.unsqueeze(1).to_broadcast([P, WIN, WIN]),
                        op=OP.mult)
                else:
                    # x-major per tile: kr[p, j, x, w] = v_j[p,x] * u_j[p,w]
                    kr4v = kr4[:, :nb * WIN2].rearrange(
                        "p (j x w) -> p j x w", x=WIN, w=WIN)
                    wv = wuv[:, t0 * P:(t0 + nb) * P].rearrange(
                        "p (j c) -> p j c", c=P)
                    u4 = wv[:, :, UOFF:UOFF + WIN]
                    v4 = wv[:, :, VOFF:VOFF + WIN]
                    eng.tensor_tensor(
                        out=kr4v[:],
                        in0=v4.unsqueeze(3).to_broadcast([P, nb, WIN, WIN]),
                        in1=u4.unsqueeze(2).to_broadcast([P, nb, WIN, WIN]),
                        op=OP.mult)
                kr_groups[g] = kr4
            return kr_groups[g], (t - g * KR_BATCH) * WIN2

        vol3 = vol[:].rearrange("p (y x) -> p y x", x=XSTRIDE)
        out_lo = [0]      # next interior y row to DMA out

        def frags(r0, r1):
            # 64-aligned (t, p0, p1) segments (base partition 0/64)
            out = []
            r = r0
            while r < r1:
                t = r // P
                p0 = r - t * P
                p1 = min(r1 - t * P, P)
                out.append((t, p0, p1))
                r = t * P + p1
            return out

        # per-pair slabs, one PSUM bank each; group structure from pair_plan
        # (consecutive matmuls in a group have intersecting partition
        # ranges, else the HW runs them concurrently on one PSUM bank)
        BOFF = BINV * WIN
        pair_plan = bins_meta["pair_plan"]
        for by in range(ROWS_EMIT):
            y0 = by * BINV
            # interleaved x-pair order: consecutive drains touch disjoint
            # vol regions, breaking the margin-overlap WAR chain
            for bp in (0, 2, 1, 3):
                b0 = by * NB + 2 * bp
                x0 = 2 * bp * BINV
                for group in pair_plan[by * 4 + bp]:
                    slab = slabp.tile([BOX, SLABC], F32, tag="slab", bufs=4)
                    solo = len(group) == 1
                    mms = []
                    for loc in group:
                        olo = 0 if (solo or loc == 0) else BOFF
                        for (t, p0, p1) in frags(int(bin_r0[b0 + loc]),
                                                 int(bin_r0[b0 + loc + 1])):
                            mms.append((t, p0, p1, olo, olo + WIN2))
                    for i, (t, p0, p1, olo, ohi) in enumerate(mms):
                        kr4, klo = get_kr(t)
                        lhsT = wuv[p0:p1, ts(t, P)][:, :BOX]
                        nc.tensor.matmul(out=slab[:, olo:ohi],
                                         lhsT=lhsT,
                                         rhs=kr4[p0:p1, klo:klo + WIN2],
                                         start=(i == 0),
                                         stop=(i == len(mms) - 1),
                                         skip_group_check=True)

                    if solo:
                        xg = x0 + group[0] * BINV
                        wdt = WIN
                    else:
                        xg = x0
                        wdt = XPAIRW
                    cols = wdt * WIN
                    dst = vol3[:, y0:y0 + WIN, xg:xg + wdt]
                    dve_new = max(load["dve"] + C_DR_DVE + cols * 1.042,
                                  load["pool"], load["act"])
                    via_new = max(load["dve"],
                                  load["pool"] + C_DR_POOL + cols * 1.98,
                                  load["act"] + C_DR_ACT + cols * 0.833)
                    if dve_new <= via_new:
                        load["dve"] += C_DR_DVE + cols * 1.042
                        eng, src = nc.vector, slab
                    else:
                        load["pool"] += C_DR_POOL + cols * 1.98
                        load["act"] += C_DR_ACT + cols * 0.833
                        tmp = wk.tile([BOX, SLABC], BF16, tag="drt", bufs=4)
                        nc.scalar.copy(tmp[:, :cols], slab[:, :cols])
                        eng, src = nc.gpsimd, tmp
                    # slab is x-major [p, x, w]; drain wants [p, w(y), x]
                    srcwx = src[:, :cols].rearrange("p (x w) -> p w x", w=WIN)
                    eng.tensor_tensor(out=dst, in0=dst, in1=srcwx, op=OP.add)
            # stale kr handles may alias recycled pool buffers across rows
            kr_groups.clear()
            # interior y rows < (by+1)*BINV - 1 are final; stream them out
            if OUT_CHUNKS or by == NB - 1:
                hi = (by + 1) * BINV - 1 if by < NB - 1 else BOX
                if not OUT_CHUNKS:
                    hi = BOX
                lo = out_lo[0]
                if hi > lo:
                    src = vol3[:, 1 + lo:1 + hi, MARGIN:MARGIN + BOX]
                    nc.sync.dma_start(
                        vol_d[:, lo * BOX:hi * BOX], src)
                    out_lo[0] = hi


# ---------------------------------------------------------------------------
# device program k2: sum partial z-shards + reduce
# ---------------------------------------------------------------------------

def _build_k2(loop_reps=0, unroll=1):
    COLF = K2_COLF
    nc = bacc.Bacc("TRN2", target_bir_lowering=False, debug=False,
                   num_devices=N_CORES)
    vst_d = nc.dram_tensor("vstack", [BOX, N_CORES * COLF], BF16,
                           kind="ExternalInput")
    g2_d = nc.dram_tensor("g2", [BOX, COLF], BF16, kind="ExternalInput")
    out_d = nc.dram_tensor("res2", [2 * K2_CH, 1], F32, kind="ExternalOutput")
    ones_np = np.ones((BOX, 1), np.float32)
    ones_c = nc.inline_tensor(ones_np, name="ones_c")

    with tile.TileContext(nc) as tc:
        with tc.tile_pool(name="p", bufs=1) as p, \
             tc.tile_pool(name="ps", bufs=1, space="PSUM") as ps:
            vst = p.tile([BOX, N_CORES * COLF], BF16)
            g2 = p.tile([BOX, COLF], BF16)
            ones = p.tile([BOX, 1], F32)
            nc.sync.dma_start(ones[:], ones_c[:, :])

            def body():
                parts = p.tile([BOX, 2 * K2_CH], F32, tag="parts")
                b0 = 0
                f0 = 0
                for c, cw in enumerate(K2_CWS):
                    cws = N_CORES * cw
                    # chunk DMA split by slot-pairs: first-level tree adds
                    # start as soon as their pair lands (chunk-major layout)
                    for sp in range(4):
                        lo = b0 + sp * 2 * cw
                        nc.sync.dma_start(vst[:, lo:lo + 2 * cw],
                                          vst_d[:, lo:lo + 2 * cw])
                    nc.sync.dma_start(g2[:, f0:f0 + cw],
                                      g2_d[:, f0:f0 + cw])
                    vc = vst[:, b0:b0 + cws].rearrange(
                        "p (s f) -> p s f", f=cw)
                    eng = nc.gpsimd if c == 0 else nc.vector
                    # pairwise tree sum into source slot 0 region (bf16)
                    for step in (1, 2, 4):
                        for s0 in range(0, N_CORES, 2 * step):
                            eng.tensor_tensor(
                                out=vc[:, s0, :], in0=vc[:, s0, :],
                                in1=vc[:, s0 + step, :], op=OP.add)
                    vsum = vc[:, 0, :]
                    prod = p.tile([BOX, cw], BF16, tag=f"prod{c}")
                    # fused mult + row-sum via scalar_tensor_tensor accum_out
                    # (DVE only — Pool rejects TensorScalarPtr at codegen)
                    nc.vector.scalar_tensor_tensor(
                        out=prod[:], in0=vsum, scalar=1.0, in1=vsum,
                        op0=OP.mult, op1=OP.mult,
                        accum_out=parts[:, 2 * c:2 * c + 1])
                    nc.vector.scalar_tensor_tensor(
                        out=prod[:], in0=vsum, scalar=1.0, in1=g2[:, f0:f0 + cw],
                        op0=OP.mult, op1=OP.mult,
                        accum_out=parts[:, 2 * c + 1:2 * c + 2])
                    b0 += cws
                    f0 += cw
                red = ps.tile([2 * K2_CH, 1], F32, tag="red")
                nc.tensor.matmul(out=red[:], lhsT=parts[:, :], rhs=ones[:, :],
                                 start=True, stop=True)
                red_sb = p.tile([2 * K2_CH, 1], F32, tag="red_sb")
                nc.scalar.copy(red_sb[:], red[:])
                return red_sb

            if loop_reps:
                with tc.For_i(0, loop_reps, 1):
                    red_sb = body()
            else:
                for _ in range(unroll):
                    red_sb = body()
            nc.sync.dma_start(out_d[:, :], red_sb[:])

    nc.compile()
    return nc


# ---------------------------------------------------------------------------
# entry
# ---------------------------------------------------------------------------

def kernel(quat, offset, positions, amplitudes, variances, voxel_grid):
    bins_meta, in_maps, c0, g = _prepare(
        quat, offset, positions, amplitudes, variances, voxel_grid)
    key = ("k1", bins_meta["T"], tuple(bins_meta["bin_r0"].tolist()))
    if key not in _cache:
        _cache[key] = _build_k1(bins_meta)
    nc1 = _cache[key]
    res1 = run_bass_kernel_spmd(nc1, in_maps, core_ids=list(range(N_CORES)))
    vols = [res1.results[c]["vol"] for c in range(N_CORES)]

    in2 = _prepare_k2(c0, g, vols)
    if "k2" not in _cache:
        _cache["k2"] = _build_k2()
    nc2 = _cache["k2"]
    res2 = run_bass_kernel_spmd(nc2, in2, core_ids=list(range(N_CORES)))

    ssq = dot = 0.0
    for c in range(N_CORES):
        r = res2.results[c]["res2"]
        for ch in range(K2_CH):
            ssq += float(r[2 * ch, 0])
            dot += float(r[2 * ch + 1, 0])
    gssq = float((g.astype(np.float64) ** 2).sum())
    corr = dot / math.sqrt(ssq * gssq)
    return np.float32(1.0 - corr)


# revision 44
# speedup vs baseline: 1.2819x; 1.0384x over previous
"""Trainium2 Bass kernel for ModelToVolumeAligner — v3 (2D-binned, collective-free).

Strategy:
  - Host: rotate positions, clip to an 88^3 box centered on the cloud,
    2D-bin atoms by (y,x) into 8x8 bins of 11 voxels, shard atoms per bin
    round-robin across 8 cores (uniform compile-time layout, 64-aligned
    per-bin rank ranges), precompute per-rank quadratic exp-arg params.
  - Device k1 (per core): per 128-rank tile, ONE small matmul (contract
    18 f16 hi/lo rows) against a fixed basis computes the exp-arguments
    for the z-profile W (88 cols) and the y/x window profiles u,v
    (13 cols each, MARGIN=1); batched ACT Exp over 8-tile chunks -> bf16.
    kr = u (x) v via 4-tile-batched broadcast TT on Pool or DVE (greedy
    load balance). Splat: matmul contracting ranks: slab[z, 13x13 win]
    += W^T @ kr, PSUM-accumulated per x-pair group; drained into a bf16
    vol [88, 90y x 112-stride] on DVE or via ACT copy + Pool add (greedy).
    Interior [88,88,88] DMA'd out in y-chunks as each bin-row completes.
  - HW pitfall baked into the layout (_prepare pair_plan search): two
    matmuls accumulating into the SAME PSUM bank whose partition ranges
    are DISJOINT execute concurrently (per-subarray LDWEIGHTS pull-ahead)
    and fault the device.  Consecutive matmuls of a group must intersect
    in partitions; pairs that can't be ordered that way are split into
    solo-bin groups (own slab + drain) or padded by one 64-unit.
  - Host: restack 8 partial volumes into per-core z-shards.
  - Device k2 (per core): 2 column chunks (Pool small, DVE big),
    contiguous chunk DMAs split by slot-pairs; pairwise-tree add; fused
    ssq/dot row-reductions via DVE scalar_tensor_tensor accum_out
    (Pool rejects TensorScalarPtr); PE ones-matmul -> [2*CH] scalars.
  - Host: combine scalars + host gssq -> 1 - dot/sqrt(ssq*gssq).
    (The v/v.sum() normalization cancels in the correlation.)
  - No collectives: ReduceScatter in this environment costs ~1ms fixed.
"""

import math
import numpy as np

import concourse.bass as bass
import concourse.mybir as mybir
import concourse.tile as tile
from concourse import bacc
from concourse.bass import ts
from concourse.bass_utils import run_bass_kernel_spmd

F32 = mybir.dt.float32
F16 = mybir.dt.float16
BF16 = mybir.dt.bfloat16
AF = mybir.ActivationFunctionType
OP = mybir.AluOpType

N_CORES = 8
P = 128
G = 5
BOX = 88            # clipped cube side (voxels)
NB = 8              # bins per axis
BINV = BOX // NB    # 11 voxels per bin
MARGIN = 1
WIN = BINV + 2 * MARGIN      # 13
WIN2 = WIN * WIN             # 169
KC = BOX // 2                # z-basis centering (44)
KU = WIN // 2                # window-basis centering (6)
YPAD = BOX + 2 * MARGIN      # 90
XSTRIDE = 112                # padded x-row stride (fits 2-pair drain views)
VOLC = YPAD * XSTRIDE        # 10080
ZSH = BOX // N_CORES         # 11
XPAIRW = WIN + BINV          # 24 (x-paired slab width)
SLABC = WIN * XPAIRW         # 312
PADW = -60.0                 # pad-rank W arg constant (exp -> 0)

NPAR = 18                    # f16 hi/lo split of 9 quadratic params
ARG_COLS = 128               # basis tile width (114 used)
TB = BOX + 2 * WIN           # 114 used cols per tile block
UOFF = BOX                   # u cols offset within tile block
VOFF = BOX + WIN
CHUNK_T = 8                  # tiles per arg/exp chunk
ARGP_BUFS = 2                # in-flight arg chunks (PSUM banks)
LOOKAHEAD = 2                # arg chunks emitted ahead of kr consumption
KR_BATCH = 4                 # tiles per kr TT instruction (1 = per-tile 2D)
DRAIN_BATCH = True           # drain both couple pairs in one 3-free-dim TT
OUT_CHUNKS = True            # stream interior vol out per y-row of bins
ROWS_EMIT = NB               # debug: emit only the first k bin-rows
DMA_SPLIT = 4
BIN_ALIGN = 64               # bin rank-range alignment (PE base partition 0/64)
K2_CH = 2                    # k2 column chunks

# engine cost constants (ns) for greedy balancing, calibrated against
# timeline-sim slice durations of this exact build:
#   Pool broadcast-TT ~2.0 ns/el + 95 launch; DVE TT 1.042 ns/el + inits;
#   ACT copy 0.833 ns/el + 186; Pool SBUF add 1.39 ns/el + 95.
C_KR_POOL_FIX, C_KR_POOL_EL = 95.0, 1.98 * WIN2
C_KR_DVE_FIX, C_KR_DVE_EL = 60.0, 1.042 * WIN2
C_DR_DVE = 125.0             # + cols*1.042 (PSUM access latency on DVE)
C_DR_ACT = 186.0             # + cols*0.833 (copy PSUM->SBUF)
C_DR_POOL = 95.0             # + cols*1.98 (SBUF add)
C_ACT_CHUNK_FIX = 130.0

_cache = {}


def _rotmat(quat):
    q = quat.astype(np.float64)
    q = q / np.sqrt((q * q).sum())
    w, x, y, z = q
    return np.array(
        [
            [1 - 2 * (y * y + z * z), 2 * (x * y - w * z), 2 * (x * z + w * y)],
            [2 * (x * y + w * z), 1 - 2 * (x * x + z * z), 2 * (y * z - w * x)],
            [2 * (x * z - w * y), 2 * (y * z + w * x), 1 - 2 * (x * x + y * y)],
        ],
        dtype=np.float32,
    )


# ---------------------------------------------------------------------------
# host-side prep
# ---------------------------------------------------------------------------

def _prepare(quat, offset, positions, amplitudes, variances, voxel_grid):
    quat = np.asarray(quat, np.float32)
    offset = np.asarray(offset, np.float32)
    positions = np.asarray(positions, np.float32)
    amplitudes = np.asarray(amplitudes, np.float32)
    variances = np.asarray(variances, np.float32)
    voxel_grid = np.asarray(voxel_grid, np.float32)

    rot = _rotmat(quat)
    pos = positions @ rot + offset          # [A,3], voxel units, center=idx 64
    A = positions.shape[0]

    # box origin per axis (global grid index of box cell 0)
    c0 = np.round(offset).astype(np.int64) + 64 - BOX // 2   # [3] (x,y,z order)
    c0 = np.clip(c0, 0, 128 - BOX)
    pb = pos + 64.0 - c0[None, :].astype(np.float32)          # box coords [A,3]
    px, py, pz = pb[:, 0], pb[:, 1], pb[:, 2]

    by = np.clip(np.floor(py / BINV).astype(np.int64), 0, NB - 1)
    bx = np.clip(np.floor(px / BINV).astype(np.int64), 0, NB - 1)
    bin_id = by * NB + bx
    order = np.argsort(bin_id, kind="stable")

    # per-bin atom lists (global)
    counts = np.bincount(bin_id, minlength=NB * NB)
    starts = np.concatenate([[0], np.cumsum(counts)])
    cap = np.ceil(counts / N_CORES).astype(np.int64)     # per-core padded count
    # rank ranges per bin, padded to BIN_ALIGN units
    units = np.ceil(cap * G / BIN_ALIGN).astype(np.int64)   # 64-rank units

    # HW constraint: two in-flight matmuls on the same PSUM bank with
    # DISJOINT partition ranges execute concurrently (per-subarray
    # LDWEIGHTS pull-ahead) and fault.  Within each accumulation group,
    # consecutive matmul frags must therefore INTERSECT in partitions.
    # Per pair, search emit orders / solo-split / padding bumps.
    def fragseq(phi, n):
        # frag partition intervals for n 64-units starting at 64-phase phi
        out = []
        r = phi
        end = phi + n
        while r < end:
            t = r // 2
            p0 = (r - t * 2) * 64
            p1 = min(end - t * 2, 2) * 64
            out.append((p0, p1))
            r = t * 2 + min(end - t * 2, 2)
        return out

    def seq_ok(seq):
        return all(not (a[1] <= b[0] or b[1] <= a[0])
                   for a, b in zip(seq, seq[1:]))

    pair_plan = []    # per pair: list of groups, each = list of local bin idx
    units = units.copy()
    phi = 0
    for pr in range(NB * NB // 2):
        b0 = 2 * pr
        for bump in range(4):
            a, b = int(units[b0]), int(units[b0 + 1])
            f0 = fragseq(phi, a)
            f1 = fragseq(phi + a, b)
            found = None
            if seq_ok(f0 + f1):
                found = [[0, 1]]
            elif seq_ok(f1 + f0):
                found = [[1, 0]]
            elif seq_ok(f0) and seq_ok(f1):
                found = [[0], [1]]
            if found is not None:
                pair_plan.append(found)
                break
            # bump the smaller bin by one unit and retry
            if a <= b:
                units[b0] += 1
            else:
                units[b0 + 1] += 1
        else:
            raise RuntimeError("layout fixer failed")
        phi += int(units[b0] + units[b0 + 1])

    slots = units * BIN_ALIGN
    bin_r0 = np.concatenate([[0], np.cumsum(slots)])
    R_real = int(bin_r0[-1])
    T = (R_real + P - 1) // P
    R_pad = T * P

    # per-core rank -> atom mapping
    atom_of = np.full((N_CORES, R_pad), -1, np.int64)
    g_of = np.zeros((N_CORES, R_pad), np.int64)
    for b in range(NB * NB):
        ix = order[starts[b]:starts[b + 1]]
        for c in range(N_CORES):
            mine = ix[c::N_CORES]
            n = len(mine)
            if n == 0:
                continue
            r0 = int(bin_r0[b])
            rr = r0 + np.arange(n * G)
            atom_of[c, rr] = np.repeat(mine, G)
            g_of[c, rr] = np.tile(np.arange(G), n)

    # params
    in_maps = []
    for c in range(N_CORES):
        av = atom_of[c]
        valid = av >= 0
        a_ = np.where(valid, av, 0)
        g_ = g_of[c]
        var_r = variances[a_, g_]
        amp_r = amplitudes[a_, g_]
        sc = (-0.5 / var_r).astype(np.float32)
        lnpref = (np.log(amp_r) - 1.5 * np.log(2 * np.pi * var_r)).astype(np.float32)

        pzr = pz[a_].astype(np.float32)
        pyr = py[a_].astype(np.float32)
        pxr = px[a_].astype(np.float32)
        # bin of each rank (recompute from layout for pad safety)
        rb = np.searchsorted(bin_r0[1:], np.arange(R_pad), side="right")
        rb = np.minimum(rb, NB * NB - 1)
        rby, rbx = rb // NB, rb % NB
        dy = pyr - (rby * BINV - MARGIN) - KU     # centered window offset
        dx = pxr - (rbx * BINV - MARGIN) - KU
        zc = pzr - KC

        pars = np.zeros((9, R_pad), np.float32)
        pars[0] = sc
        pars[1] = -2 * sc * zc
        pars[2] = sc * zc * zc + lnpref
        pars[3] = sc
        pars[4] = -2 * sc * dy
        pars[5] = sc * dy * dy
        pars[6] = sc
        pars[7] = -2 * sc * dx
        pars[8] = sc * dx * dx
        # pad ranks: W=0 (exp(PADW)), u=v=1
        pars[:, ~valid] = 0.0
        pars[2, ~valid] = PADW

        def split16(x):
            hi = x.astype(np.float16)
            lo = (x - hi.astype(np.float32)).astype(np.float16)
            return hi, lo
        p18 = np.zeros((NPAR, R_pad), np.float16)
        for i in range(9):
            p18[2 * i], p18[2 * i + 1] = split16(pars[i])
        in_maps.append({"pars": p18})

    bins_meta = {
        "bin_r0": bin_r0.astype(np.int64),
        "T": T,
        "R_pad": R_pad,
        "counts": counts,
        "cap": cap,
        "pair_plan": pair_plan,
    }
    return bins_meta, in_maps, c0, voxel_grid


K2_COLF = BOX * BOX // N_CORES            # 968
K2_CWS = (208, K2_COLF - 208)             # chunk widths: Pool small, DVE big


def _prepare_k2(c0, voxel_grid, vols):
    """vols: list of 8 partial interior volumes [BOX, BOX*BOX] bf16."""
    COLF = K2_COLF
    gz0, gy0, gx0 = int(c0[2]), int(c0[1]), int(c0[0])
    in_maps = []
    for c in range(N_CORES):
        # vstack chunk-major [88, (chunk, 8, cw)]: chunk DMAs are contiguous
        vst = np.zeros((BOX, N_CORES * COLF), dtype=vols[0].dtype)
        for p in range(N_CORES):
            sl = vols[p][ZSH * c:ZSH * (c + 1)]          # [11, 7744]
            s3 = sl.reshape(ZSH, N_CORES, COLF)          # [11, 8ch, 968]
            vst[:, p * COLF:(p + 1) * COLF] = s3.reshape(BOX, COLF)
        v4 = vst.reshape(BOX, N_CORES, COLF)
        parts = []
        f0 = 0
        for cw in K2_CWS:
            parts.append(v4[:, :, f0:f0 + cw].reshape(BOX, -1))
            f0 += cw
        vst = np.concatenate(parts, axis=1)
        gsl = voxel_grid[gz0 + ZSH * c: gz0 + ZSH * (c + 1),
                         gy0:gy0 + BOX, gx0:gx0 + BOX]   # [11, 88, 88] f32
        g2 = gsl.reshape(ZSH, N_CORES, COLF).reshape(BOX, COLF)
        import ml_dtypes
        g2b = np.ascontiguousarray(g2, np.float32).astype(ml_dtypes.bfloat16)
        in_maps.append({"vstack": vst, "g2": g2b})
    return in_maps


# ---------------------------------------------------------------------------
# device program k1: splat partial volume
# ---------------------------------------------------------------------------

def _build_k1(bins_meta, loop_reps=0, unroll=1):
    T = bins_meta["T"]
    R_pad = bins_meta["R_pad"]

    nc = bacc.Bacc("TRN2", target_bir_lowering=False, debug=False,
                   num_devices=N_CORES)
    pars_d = nc.dram_tensor("pars", [NPAR, R_pad], F16, kind="ExternalInput")
    # interior volume only (host restacks directly)
    vol_d = nc.dram_tensor("vol", [BOX, BOX * BOX], BF16, kind="ExternalOutput")

    # basis constants
    bas9 = np.zeros((9, ARG_COLS), np.float32)
    zc = np.arange(BOX, dtype=np.float32) - KC
    bas9[0, :BOX] = zc * zc
    bas9[1, :BOX] = zc
    bas9[2, :BOX] = 1.0
    wc = np.arange(WIN, dtype=np.float32) - KU
    bas9[3, UOFF:UOFF + WIN] = wc * wc
    bas9[4, UOFF:UOFF + WIN] = wc
    bas9[5, UOFF:UOFF + WIN] = 1.0
    bas9[6, VOFF:VOFF + WIN] = wc * wc
    bas9[7, VOFF:VOFF + WIN] = wc
    bas9[8, VOFF:VOFF + WIN] = 1.0
    bas_np = np.repeat(bas9, 2, axis=0).astype(np.float16)
    bas_c = nc.inline_tensor(bas_np, name="bas_c")

    with tile.TileContext(nc) as tc:
        with tc.tile_pool(name="keep", bufs=1) as keep:
            bas = keep.tile([NPAR, ARG_COLS], F16)
            nc.sync.dma_start(bas[:], bas_c[:, :])
            pars = keep.tile([NPAR, R_pad], F16)
            wuv = keep.tile([P, T * P], BF16)
            vol = keep.tile([BOX, VOLC], BF16)

            # one-time zeros in DRAM; per-rep vol clear is then DMA-only
            zdp = tc.tile_pool(name="zd", bufs=1, space="DRAM")
            zd = zdp.__enter__()
            zer_d = zd.tile([BOX, VOLC], BF16)
            quart = VOLC // 4
            nc.vector.memset(vol[:, :quart], 0.0)
            nc.vector.memset(vol[:, quart:2 * quart], 0.0)
            nc.gpsimd.memset(vol[:, 2 * quart:3 * quart], 0.0)
            nc.gpsimd.memset(vol[:, 3 * quart:], 0.0)
            cwz = VOLC // DMA_SPLIT
            for s in range(DMA_SPLIT):
                nc.sync.dma_start(zer_d[:, s * cwz:(s + 1) * cwz],
                                  vol[:, s * cwz:(s + 1) * cwz])

            def body():
                cw = R_pad // DMA_SPLIT
                for s in range(DMA_SPLIT):
                    nc.sync.dma_start(pars[:, s * cw:(s + 1) * cw],
                                      pars_d[:, s * cw:(s + 1) * cw])
                _k1_body(nc, tc, bins_meta, bas, pars, wuv, vol, zer_d, vol_d)

            if loop_reps:
                with tc.For_i(0, loop_reps, 1):
                    body()
            else:
                for _ in range(unroll):
                    body()
            zdp.__exit__(None, None, None)

    nc.compile()
    return nc


def _k1_body(nc, tc, bins_meta, bas, pars, wuv, vol, zer_d, vol_d):
    T = bins_meta["T"]
    bin_r0 = bins_meta["bin_r0"]
    NCHUNK = (T + CHUNK_T - 1) // CHUNK_T

    # vol clear via DMA from the DRAM zeros buffer (no engine time)
    cwz = VOLC // DMA_SPLIT
    for s in range(DMA_SPLIT):
        nc.sync.dma_start(vol[:, s * cwz:(s + 1) * cwz],
                          zer_d[:, s * cwz:(s + 1) * cwz])

    # greedy engine load tracker (ns); ACT preloaded with fixed chunk exps
    load = {"dve": 0.0, "pool": 0.0,
            "act": NCHUNK * (CHUNK_T * TB * 1.03 + C_ACT_CHUNK_FIX)}

    with tc.tile_pool(name="work", bufs=8) as wk, \
         tc.tile_pool(name="argp", bufs=ARGP_BUFS, space="PSUM") as argp, \
         tc.tile_pool(name="slabp", bufs=2, space="PSUM") as slabp:

        def emit_chunk(cc):
            if cc >= NCHUNK:
                return
            t0 = cc * CHUNK_T
            n = min(CHUNK_T, T - t0)
            ac = argp.tile([P, CHUNK_T * ARG_COLS], F32, tag="ac",
                           bufs=ARGP_BUFS)
            for j in range(n):
                t = t0 + j
                nc.tensor.matmul(out=ac[:, j * ARG_COLS:j * ARG_COLS + TB],
                                 lhsT=pars[:, ts(t, P)], rhs=bas[:, :TB],
                                 start=True, stop=True)
            ac3 = ac[:].rearrange("p (j c) -> p j c", c=ARG_COLS)
            wv3 = wuv[:, t0 * P:(t0 + n) * P].rearrange(
                "p (j c) -> p j c", c=P)
            nc.scalar.activation(wv3[:, :, :TB], ac3[:, :n, :TB], AF.Exp)

        # software pipeline: keep arg chunks ahead of splat tiles
        for cc in range(LOOKAHEAD):
            emit_chunk(cc)

        kr_groups = {}    # g -> kr4 tile handle (tiles 4g..4g+3 batched)
        emitted = [LOOKAHEAD]

        def get_kr(t):
            g = t // KR_BATCH
            if g not in kr_groups:
                t0 = g * KR_BATCH
                nb = min(KR_BATCH, T - t0)
                # pipeline arg chunks past the whole batch
                while emitted[0] <= (t0 + nb - 1) // CHUNK_T + LOOKAHEAD:
                    emit_chunk(emitted[0])
                    emitted[0] += 1
                c_pool = C_KR_POOL_FIX + nb * C_KR_POOL_EL
                c_dve = C_KR_DVE_FIX + nb * C_KR_DVE_EL
                if load["pool"] + c_pool <= load["dve"] + c_dve:
                    eng = nc.gpsimd
                    load["pool"] += c_pool
                else:
                    eng = nc.vector
                    load["dve"] += c_dve
                kr4 = wk.tile([P, KR_BATCH * WIN2], BF16, tag="kr", bufs=8)
                if KR_BATCH == 1:
                    kr3 = kr4[:].rearrange("p (x w) -> p x w", w=WIN)
                    u = wuv[:, t0 * P + UOFF:t0 * P + UOFF + WIN]
                    v = wuv[:, t0 * P + VOFF:t0 * P + VOFF + WIN]
                    eng.tensor_tensor(
                        out=kr3[:],
                        in0=v.unsqueeze(2).to_broadcast([P, WIN, WIN]),
                        in1=u.unsqueeze(1).to_broadcast([P, WIN, WIN]),
                        op=OP.mult)
                else:
                    # x-major per tile: kr[p, j, x, w] = v_j[p,x] * u_j[p,w]
                    kr4v = kr4[:, :nb * WIN2].rearrange(
                        "p (j x w) -> p j x w", x=WIN, w=WIN)
                    wv = wuv[:, t0 * P:(t0 + nb) * P].rearrange(
                        "p (j c) -> p j c", c=P)
                    u4 = wv[:, :, UOFF:UOFF + WIN]
                    v4 = wv[:, :, VOFF:VOFF + WIN]
                    eng.tensor_tensor(
                        out=kr4v[:],
                        in0=v4.unsqueeze(3).to_broadcast([P, nb, WIN, WIN]),
                        in1=u4.unsqueeze(2).to_broadcast([P, nb, WIN, WIN]),
                        op=OP.mult)
                kr_groups[g] = kr4
            return kr_groups[g], (t - g * KR_BATCH) * WIN2

        vol3 = vol[:].rearrange("p (y x) -> p y x", x=XSTRIDE)
        out_lo = [0]      # next interior y row to DMA out

        def frags(r0, r1):
            # 64-aligned (t, p0, p1) segments (base partition 0/64)
            out = []
            r = r0
            while r < r1:
                t = r // P
                p0 = r - t * P
                p1 = min(r1 - t * P, P)
                out.append((t, p0, p1))
                r = t * P + p1
            return out

        # per-pair slabs, one PSUM bank each; group structure from pair_plan
        # (consecutive matmuls in a group have intersecting partition
        # ranges, else the HW runs them concurrently on one PSUM bank)
        BOFF = BINV * WIN
        pair_plan = bins_meta["pair_plan"]
        for by in range(ROWS_EMIT):
            y0 = by * BINV
            # interleaved x-pair order: consecutive drains touch disjoint
            # vol regions, breaking the margin-overlap WAR chain
            for bp in (0, 2, 1, 3):
                b0 = by * NB + 2 * bp
                x0 = 2 * bp * BINV
                for group in pair_plan[by * 4 + bp]:
                    slab = slabp.tile([BOX, SLABC], F32, tag="slab", bufs=4)
                    solo = len(group) == 1
                    mms = []
                    for loc in group:
                        olo = 0 if (solo or loc == 0) else BOFF
                        for (t, p0, p1) in frags(int(bin_r0[b0 + loc]),
                                                 int(bin_r0[b0 + loc + 1])):
                            mms.append((t, p0, p1, olo, olo + WIN2))
                    for i, (t, p0, p1, olo, ohi) in enumerate(mms):
                        kr4, klo = get_kr(t)
                        lhsT = wuv[p0:p1, ts(t, P)][:, :BOX]
                        nc.tensor.matmul(out=slab[:, olo:ohi],
                                         lhsT=lhsT,
                                         rhs=kr4[p0:p1, klo:klo + WIN2],
                                         start=(i == 0),
                                         stop=(i == len(mms) - 1),
                                         skip_group_check=True)

                    if solo:
                        xg = x0 + group[0] * BINV
                        wdt = WIN
                    else:
                        xg = x0
                        wdt = XPAIRW
                    cols = wdt * WIN
                    dst = vol3[:, y0:y0 + WIN, xg:xg + wdt]
                    dve_new = max(load["dve"] + C_DR_DVE + cols * 1.042,
                                  load["pool"], load["act"])
                    via_new = max(load["dve"],
                                  load["pool"] + C_DR_POOL + cols * 1.98,
                                  load["act"] + C_DR_ACT + cols * 0.833)
                    if dve_new <= via_new:
                        load["dve"] += C_DR_DVE + cols * 1.042
                        eng, src = nc.vector, slab
                    else:
                        load["pool"] += C_DR_POOL + cols * 1.98
                        load["act"] += C_DR_ACT + cols * 0.833
                        tmp = wk.tile([BOX, SLABC], BF16, tag="drt", bufs=4)
                        nc.scalar.copy(tmp[:, :cols], slab[:, :cols])
                        eng, src = nc.gpsimd, tmp
                    # slab is x-major [p, x, w]; drain wants [p, w(y), x]
                    srcwx = src[:, :cols].rearrange("p (x w) -> p w x", w=WIN)
                    eng.tensor_tensor(out=dst, in0=dst, in1=srcwx, op=OP.add)
            # stale kr handles may alias recycled pool buffers across rows
            kr_groups.clear()
            # interior y rows < (by+1)*BINV - 1 are final; stream them out
            if OUT_CHUNKS or by == NB - 1:
                hi = (by + 1) * BINV - 1 if by < NB - 1 else BOX
                if not OUT_CHUNKS:
                    hi = BOX
                lo = out_lo[0]
                if hi > lo:
                    src = vol3[:, 1 + lo:1 + hi, MARGIN:MARGIN + BOX]
                    nc.sync.dma_start(
                        vol_d[:, lo * BOX:hi * BOX], src)
                    out_lo[0] = hi


# ---------------------------------------------------------------------------
# device program k2: sum partial z-shards + reduce
# ---------------------------------------------------------------------------

def _build_k2(loop_reps=0, unroll=1):
    COLF = K2_COLF
    nc = bacc.Bacc("TRN2", target_bir_lowering=False, debug=False,
                   num_devices=N_CORES)
    vst_d = nc.dram_tensor("vstack", [BOX, N_CORES * COLF], BF16,
                           kind="ExternalInput")
    g2_d = nc.dram_tensor("g2", [BOX, COLF], BF16, kind="ExternalInput")
    out_d = nc.dram_tensor("res2", [2 * K2_CH, 1], F32, kind="ExternalOutput")
    ones_np = np.ones((BOX, 1), np.float32)
    ones_c = nc.inline_tensor(ones_np, name="ones_c")

    with tile.TileContext(nc) as tc:
        with tc.tile_pool(name="p", bufs=1) as p, \
             tc.tile_pool(name="ps", bufs=1, space="PSUM") as ps:
            vst = p.tile([BOX, N_CORES * COLF], BF16)
            g2 = p.tile([BOX, COLF], BF16)
            ones = p.tile([BOX, 1], F32)
            nc.sync.dma_start(ones[:], ones_c[:, :])

            def body():
                parts = p.tile([BOX, 2 * K2_CH], F32, tag="parts")
                b0 = 0
                f0 = 0
                for c, cw in enumerate(K2_CWS):
                    cws = N_CORES * cw
                    # chunk DMA split by slot-pairs: first-level tree adds
                    # start as soon as their pair lands (chunk-major layout)
                    for sp in range(4):
                        lo = b0 + sp * 2 * cw
                        nc.sync.dma_start(vst[:, lo:lo + 2 * cw],
                                          vst_d[:, lo:lo + 2 * cw])
                    nc.sync.dma_start(g2[:, f0:f0 + cw],
                                      g2_d[:, f0:f0 + cw])
                    vc = vst[:, b0:b0 + cws].rearrange(
                        "p (s f) -> p s f", f=cw)
                    eng = nc.gpsimd if c == 0 else nc.vector
                    # pairwise tree sum into source slot 0 region (bf16)
                    for step in (1, 2, 4):
                        for s0 in range(0, N_CORES, 2 * step):
                            eng.tensor_tensor(
                                out=vc[:, s0, :], in0=vc[:, s0, :],
                                in1=vc[:, s0 + step, :], op=OP.add)
                    vsum = vc[:, 0, :]
                    prod = p.tile([BOX, cw], BF16, tag=f"prod{c}")
                    # fused mult + row-sum via scalar_tensor_tensor accum_out
                    # (DVE only — Pool rejects TensorScalarPtr at codegen)
                    nc.vector.scalar_tensor_tensor(
                        out=prod[:], in0=vsum, scalar=1.0, in1=vsum,
                        op0=OP.mult, op1=OP.mult,
                        accum_out=parts[:, 2 * c:2 * c + 1])
                    nc.vector.scalar_tensor_tensor(
                        out=prod[:], in0=vsum, scalar=1.0, in1=g2[:, f0:f0 + cw],
                        op0=OP.mult, op1=OP.mult,
                        accum_out=parts[:, 2 * c + 1:2 * c + 2])
                    b0 += cws
                    f0 += cw
                red = ps.tile([2 * K2_CH, 1], F32, tag="red")
                nc.tensor.matmul(out=red[:], lhsT=parts[:, :], rhs=ones[:, :],
                                 start=True, stop=True)
                red_sb = p.tile([2 * K2_CH, 1], F32, tag="red_sb")
                nc.scalar.copy(red_sb[:], red[:])
                return red_sb

            if loop_reps:
                with tc.For_i(0, loop_reps, 1):
                    red_sb = body()
            else:
                for _ in range(unroll):
                    red_sb = body()
            nc.sync.dma_start(out_d[:, :], red_sb[:])

    nc.compile()
    return nc


# ---------------------------------------------------------------------------
# entry
# ---------------------------------------------------------------------------

def kernel(quat, offset, positions, amplitudes, variances, voxel_grid):
    bins_meta, in_maps, c0, g = _prepare(
        quat, offset, positions, amplitudes, variances, voxel_grid)
    key = ("k1", bins_meta["T"], tuple(bins_meta["bin_r0"].tolist()))
    if key not in _cache:
        _cache[key] = _build_k1(bins_meta)
    nc1 = _cache[key]
    res1 = run_bass_kernel_spmd(nc1, in_maps, core_ids=list(range(N_CORES)))
    vols = [res1.results[c]["vol"] for c in range(N_CORES)]

    in2 = _prepare_k2(c0, g, vols)
    if "k2" not in _cache:
        _cache["k2"] = _build_k2()
    nc2 = _cache["k2"]
    res2 = run_bass_kernel_spmd(nc2, in2, core_ids=list(range(N_CORES)))

    ssq = dot = 0.0
    for c in range(N_CORES):
        r = res2.results[c]["res2"]
        for ch in range(K2_CH):
            ssq += float(r[2 * ch, 0])
            dot += float(r[2 * ch + 1, 0])
    gssq = float((g.astype(np.float64) ** 2).sum())
    corr = dot / math.sqrt(ssq * gssq)
    return np.float32(1.0 - corr)
